# revision 1
# baseline (speedup 1.0000x reference)
"""Gemma2 sliding-window attention (B=1, S=4096, HID=3584, 16 Q heads / 8 KV heads,
HD=256, window 2047, tanh softcap 50) on 8 Trainium2 NeuronCores.

Sharding: tensor-parallel over heads. Core c owns Q heads (2c, 2c+1) and KV head c.
  - QKV projection computed transposed ([feature, token] layout) so Q/K land in the
    layout the scores matmul needs with zero on-device transposes. V is computed in
    [token, feature] layout for the PV matmul.
  - Scores are computed transposed ([k, q] tiles); softmax uses no max-subtraction
    (tanh softcap bounds scores to +-50 so exp cannot overflow); the denominator is
    accumulated with a ones-row matmul; masking is multiplicative post-exp with 8
    precomputed boundary masks.
  - Per-core attention outputs ([512 features, 4096 tokens], bf16) are AllGathered,
    then each core computes a 448-column slice of the output projection.
Host side: weights are pre-transposed/pre-cast to bf16, RoPE cos/sin tables are
precomputed from position_ids, outputs are concatenated along the hidden dim.
"""

import sys

if "/opt/trn_rl_repo" not in sys.path:
    sys.path.insert(0, "/opt/trn_rl_repo")

import numpy as np
import ml_dtypes

import concourse.bass as bass
import concourse.tile as tile
from concourse import bacc, mybir
from concourse.bass_utils import run_bass_kernel_spmd

# Problem constants (hardcoded per harness contract)
S = 4096
HID = 3584
NH, NKV, HD = 16, 8, 256
Q_SIZE = NH * HD          # 4096
SCALE = 256.0 ** -0.5     # 1/16
SOFTCAP = 50.0
WINDOW = 2048 - 1         # 2047
THETA = 10000.0

N_CORES = 8
QK_F = 2 * HD + HD        # 768 per-core transposed-qk features: [q_h0, q_h1, k]
HOUT = HID // N_CORES     # 448 output columns per core
KO = HID // 128           # 28 contraction subtiles for projections
TT = S // 512             # 8 token tiles of 512
F32 = mybir.dt.float32
BF16 = mybir.dt.bfloat16

# Boundary-tile diagonal offsets (q0 - 128*kt). Interior iff 128 <= off <= 1536.
MASK_OFFS = [-384, -256, -128, 0, 1664, 1792, 1920, 2048]

_NC_CACHE = {}


def _phase_a(nc, tc, qk_sb, v_sb, hidT_r, wqkT_r, wvT_r, cosT, sinT):
    """QKV projection (transposed for Q/K, straight for V) + NeoX RoPE."""
    with (
        tc.tile_pool(name="wqk", bufs=1) as wqk_pool,
        tc.tile_pool(name="wv", bufs=1) as wv_pool,
        tc.tile_pool(name="hid", bufs=2) as hid_pool,
        tc.tile_pool(name="cs", bufs=2) as cs_pool,
        tc.tile_pool(name="rope", bufs=4) as rope_pool,
        tc.tile_pool(name="psA", bufs=3, space="PSUM") as psA,
        tc.tile_pool(name="psV", bufs=2, space="PSUM") as psV,
    ):
        wqk_sb = wqk_pool.tile([128, KO, QK_F], BF16)
        nc.sync.dma_start(wqk_sb, wqkT_r)
        wv_sb = wv_pool.tile([128, KO, HD], BF16)
        nc.sync.dma_start(wv_sb, wvT_r)

        for tt in range(TT):
            tsl = bass.ts(tt, 512)
            hid_t = hid_pool.tile([128, KO, 512], BF16, name="hid_t")
            nc.sync.dma_start(hid_t, hidT_r[:, :, tsl])
            cos_t = cs_pool.tile([128, 512], F32, name="cos_t")
            nc.sync.dma_start(cos_t, cosT[:, tsl])
            sin_t = cs_pool.tile([128, 512], F32, name="sin_t")
            nc.sync.dma_start(sin_t, sinT[:, tsl])

            for pair in range(3):
                ps_a = psA.tile([128, 512], F32, name="ps_qk", tag="ps_qk")
                for ko in range(KO):
                    nc.tensor.matmul(
                        ps_a,
                        wqk_sb[:, ko, bass.ts(2 * pair, 128)],
                        hid_t[:, ko, :],
                        start=(ko == 0), stop=(ko == KO - 1),
                    )
                ps_b = psA.tile([128, 512], F32, name="ps_qk2", tag="ps_qk")
                for ko in range(KO):
                    nc.tensor.matmul(
                        ps_b,
                        wqk_sb[:, ko, bass.ts(2 * pair + 1, 128)],
                        hid_t[:, ko, :],
                        start=(ko == 0), stop=(ko == KO - 1),
                    )
                # NeoX RoPE on the (x1, x2) pair, writing bf16 into qk_sb
                t1 = rope_pool.tile([128, 512], F32, name="rp1", tag="rp")
                t2 = rope_pool.tile([128, 512], F32, name="rp2", tag="rp")
                nc.vector.tensor_mul(t1, ps_a, cos_t)
                nc.vector.tensor_mul(t2, ps_b, sin_t)
                nc.vector.tensor_sub(qk_sb[:, 2 * pair, tsl], t1, t2)
                t3 = rope_pool.tile([128, 512], F32, name="rp3", tag="rp")
                t4 = rope_pool.tile([128, 512], F32, name="rp4", tag="rp")
                nc.vector.tensor_mul(t3, ps_b, cos_t)
                nc.vector.tensor_mul(t4, ps_a, sin_t)
                nc.vector.tensor_add(qk_sb[:, 2 * pair + 1, tsl], t3, t4)

            for ts4 in range(4):
                ps_v = psV.tile([128, HD], F32, name="ps_v", tag="ps_v")
                for ko in range(KO):
                    nc.tensor.matmul(
                        ps_v,
                        hid_t[:, ko, bass.ts(ts4, 128)],
                        wv_sb[:, ko, :],
                        start=(ko == 0), stop=(ko == KO - 1),
                    )
                nc.scalar.copy(v_sb[:, tt * 4 + ts4, :], ps_v)


def _phase_b(nc, tc, qk_sb, v_sb, ones_sb, ag_ins, ag_outs, masks_r):
    """Sliding-window attention with tanh softcap; writes bf16 attnT to ag_ins.

    Token halves: qb 0..3 fill ag_ins[0], qb 4..7 fill ag_ins[1]. The first
    AllGather fires as soon as the first half is done so it overlaps the
    second half's attention compute; the second overlaps phase C's start.
    """
    with (
        tc.tile_pool(name="maskp", bufs=1) as mask_pool,
        tc.tile_pool(name="probs", bufs=8) as probs_pool,
        tc.tile_pool(name="attn", bufs=4) as attn_pool,
        tc.tile_pool(name="smalls", bufs=3) as small_pool,
        tc.tile_pool(name="psS", bufs=3, space="PSUM") as psS,
        tc.tile_pool(name="psO", bufs=2, space="PSUM") as psO,
        tc.tile_pool(name="psD", bufs=1, space="PSUM") as psD,
    ):
        mask_sb = mask_pool.tile([128, 8, 512], BF16)
        nc.sync.dma_start(mask_sb, masks_r)

        for qb in range(TT):
            for h in range(2):
                q0 = qb * 512
                qsl = bass.ts(qb, 512)
                kts = list(range(max(0, 4 * qb - 16), 4 * qb + 4))
                n = len(kts)
                po0 = psO.tile([128, 512], F32, name="po0", tag="po0")
                po1 = psO.tile([128, 512], F32, name="po1", tag="po1")
                pden = psD.tile([1, 512], F32, name="pden", tag="pden")
                probs = {}

                def scores(i, h=h, q0=q0, qsl=qsl, kts=kts, probs=probs):
                    kt = kts[i]
                    ksl = bass.ts(kt, 128)
                    ps = psS.tile([128, 512], F32, name="ps_s", tag="ps_s")
                    nc.tensor.matmul(
                        ps, qk_sb[:, 4, ksl], qk_sb[:, 2 * h, qsl],
                        start=True, stop=False,
                    )
                    nc.tensor.matmul(
                        ps, qk_sb[:, 5, ksl], qk_sb[:, 2 * h + 1, qsl],
                        start=False, stop=True,
                    )
                    pt = probs_pool.tile([128, 512], BF16, name="pt", tag="pt")
                    nc.scalar.activation(
                        ps, ps, mybir.ActivationFunctionType.Tanh,
                        scale=SCALE / SOFTCAP,
                    )
                    nc.scalar.activation(
                        pt, ps, mybir.ActivationFunctionType.Exp,
                        scale=SOFTCAP,
                    )
                    off = q0 - 128 * kt
                    if not (128 <= off <= 1536):
                        mi = MASK_OFFS.index(off)
                        nc.vector.tensor_mul(pt, pt, mask_sb[:, mi, :])
                    probs[i] = pt

                def av(i, kts=kts, n=n, po0=po0, po1=po1, pden=pden, probs=probs):
                    kt = kts[i]
                    pt = probs.pop(i)
                    st, sp = (i == 0), (i == n - 1)
                    nc.tensor.matmul(po0, v_sb[:, kt, 0:128], pt,
                                     start=st, stop=sp, skip_group_check=True)
                    nc.tensor.matmul(po1, v_sb[:, kt, 128:256], pt,
                                     start=st, stop=sp, skip_group_check=True)
                    nc.tensor.matmul(pden, ones_sb, pt,
                                     start=st, stop=sp, skip_group_check=True)

                LOOK = 3
                for i in range(min(LOOK, n)):
                    scores(i)
                for i in range(n):
                    if i + LOOK < n:
                        scores(i + LOOK)
                    av(i)

                recip = small_pool.tile([1, 512], F32, name="recip", tag="recip")
                nc.vector.reciprocal(recip, pden)
                rb = small_pool.tile([128, 512], F32, name="rb", tag="rb")
                nc.gpsimd.partition_broadcast(rb, recip)
                ao0 = attn_pool.tile([128, 512], BF16, name="ao0", tag="ao")
                ao1 = attn_pool.tile([128, 512], BF16, name="ao1", tag="ao")
                nc.vector.tensor_mul(ao0, po0, rb)
                nc.vector.tensor_mul(ao1, po1, rb)
                f0 = h * HD
                ag_in = ag_ins[qb // 4]
                c0 = q0 % 2048
                nc.sync.dma_start(ag_in[f0:f0 + 128, c0:c0 + 512], ao0)
                nc.sync.dma_start(ag_in[f0 + 128:f0 + 256, c0:c0 + 512], ao1)

            if qb == 3 or qb == TT - 1:
                half = qb // 4
                nc.gpsimd.collective_compute(
                    "AllGather",
                    mybir.AluOpType.bypass,
                    replica_groups=[list(range(N_CORES))],
                    ins=[ag_ins[half].opt()],
                    outs=[ag_outs[half].opt()],
                )


def _phase_c(nc, tc, wo_sb, ag_outs, out):
    """Output projection: out[:, 448-col slice] = attn_full.T-free matmul."""
    with (
        tc.tile_pool(name="lhs", bufs=6) as lhs_pool,
        tc.tile_pool(name="outp", bufs=4) as out_pool,
        tc.tile_pool(name="psC", bufs=8, space="PSUM") as psC,
    ):
        FO = Q_SIZE // 128  # 32
        for tg in range(TT):
            pcs = [
                psC.tile([128, HOUT], F32, name=f"pc{j}", tag="pc")
                for j in range(4)
            ]
            ag_out = ag_outs[tg // 4]
            for fo in range(FO):
                lt = lhs_pool.tile([128, 512], BF16, name="lt", tag="lt")
                nc.sync.dma_start(lt, ag_out[bass.ts(fo, 128), bass.ts(tg % 4, 512)])
                for j in range(4):
                    nc.tensor.matmul(
                        pcs[j], lt[:, bass.ts(j, 128)], wo_sb[:, fo, :],
                        start=(fo == 0), stop=(fo == FO - 1),
                        skip_group_check=True,
                    )
            for j in range(4):
                ot = out_pool.tile([128, HOUT], F32, name="ot", tag="ot")
                nc.scalar.copy(ot, pcs[j])
                nc.sync.dma_start(out[bass.ts(tg * 4 + j, 128), :], ot)


def build_nc():
    nc = bacc.Bacc()

    hidT = nc.declare_dram_parameter("hidT", [HID, S], BF16, isOutput=False)
    wqkT = nc.declare_dram_parameter("wqkT", [HID, QK_F], BF16, isOutput=False)
    wvT = nc.declare_dram_parameter("wvT", [HID, HD], BF16, isOutput=False)
    woT = nc.declare_dram_parameter("woT", [Q_SIZE, HOUT], BF16, isOutput=False)
    cosT = nc.declare_dram_parameter("cosT", [128, S], F32, isOutput=False)
    sinT = nc.declare_dram_parameter("sinT", [128, S], F32, isOutput=False)
    masks = nc.declare_dram_parameter("masks", [8, 128, 512], BF16, isOutput=False)
    out = nc.declare_dram_parameter("out", [S, HOUT], F32, isOutput=True)

    hidT_r = hidT.rearrange("(ko p) t -> p ko t", p=128)
    wqkT_r = wqkT.rearrange("(ko p) f -> p ko f", p=128)
    wvT_r = wvT.rearrange("(ko p) d -> p ko d", p=128)
    woT_r = woT.rearrange("(fo p) h -> p fo h", p=128)
    masks_r = masks.rearrange("m p q -> p m q")

    with tile.TileContext(nc) as tc:
        with (
            tc.tile_pool(name="persist", bufs=1) as persist,
            tc.tile_pool(name="dram", bufs=1, space="DRAM") as dram,
        ):
            ag_in_a = dram.tile([2 * HD, S // 2], BF16)
            ag_in_b = dram.tile([2 * HD, S // 2], BF16)
            ag_out_a = dram.tile([Q_SIZE, S // 2], BF16, addr_space="Shared")
            ag_out_b = dram.tile([Q_SIZE, S // 2], BF16, addr_space="Shared")
            ag_ins = [ag_in_a, ag_in_b]
            ag_outs = [ag_out_a, ag_out_b]

            # live across phases A+B
            qk_sb = persist.tile([128, 6, S], BF16)   # roped qT/kT rows: [h0x1,h0x2,h1x1,h1x2,kx1,kx2]
            v_sb = persist.tile([128, S // 128, HD], BF16)  # v in [token, d] layout
            ones_sb = persist.tile([128, 1], BF16)
            nc.vector.memset(ones_sb, 1.0)

            _phase_a(nc, tc, qk_sb, v_sb, hidT_r, wqkT_r, wvT_r, cosT, sinT)

            with tc.tile_pool(name="wo", bufs=1) as wo_pool:
                # prefetch o-proj weights during attention
                wo_sb = wo_pool.tile([128, Q_SIZE // 128, HOUT], BF16)
                nc.sync.dma_start(wo_sb, woT_r)

                _phase_b(nc, tc, qk_sb, v_sb, ones_sb, ag_ins, ag_outs, masks_r)
                _phase_c(nc, tc, wo_sb, ag_outs, out)

    nc.compile()
    return nc


def get_nc():
    if "nc" not in _NC_CACHE:
        _NC_CACHE["nc"] = build_nc()
    return _NC_CACHE["nc"]


def prep_in_maps(inputs):
    bf16 = ml_dtypes.bfloat16
    hs = np.asarray(inputs["hidden_states"], dtype=np.float32)
    pos = np.asarray(inputs["position_ids"]).reshape(-1).astype(np.float64)
    w_qkv = np.asarray(inputs["w_qkv"], dtype=np.float32)
    w_o = np.asarray(inputs["w_o"], dtype=np.float32)

    hidT = np.ascontiguousarray(hs.reshape(S, HID).T).astype(bf16)

    inv_freq = 1.0 / (THETA ** (np.arange(HD // 2, dtype=np.float64) * 2.0 / HD))
    ang = inv_freq[:, None] * pos[None, :]
    cosT = np.cos(ang).astype(np.float32)
    sinT = np.sin(ang).astype(np.float32)

    kk = np.arange(128)[:, None]
    qq = np.arange(512)[None, :]
    masks = np.stack(
        [((qq - kk + o >= 0) & (qq - kk + o <= WINDOW)) for o in MASK_OFFS]
    ).astype(bf16)

    in_maps = []
    for c in range(N_CORES):
        wq = w_qkv[512 * c:512 * (c + 1)]
        wk = w_qkv[Q_SIZE + HD * c:Q_SIZE + HD * (c + 1)]
        wv = w_qkv[Q_SIZE + NKV * HD + HD * c:Q_SIZE + NKV * HD + HD * (c + 1)]
        wqkT = np.ascontiguousarray(np.concatenate([wq, wk], 0).T).astype(bf16)
        wvT = np.ascontiguousarray(wv.T).astype(bf16)
        woT = np.ascontiguousarray(w_o[HOUT * c:HOUT * (c + 1)].T).astype(bf16)
        in_maps.append(
            dict(hidT=hidT, wqkT=wqkT, wvT=wvT, woT=woT,
                 cosT=cosT, sinT=sinT, masks=masks)
        )
    return in_maps


def run(inputs, **kwargs):
    nc = get_nc()
    in_maps = prep_in_maps(inputs)
    return run_bass_kernel_spmd(nc, in_maps, list(range(N_CORES)), **kwargs)


def kernel(**inputs):
    res = run(inputs)
    outs = [res.results[c]["out"] for c in range(N_CORES)]
    full = np.concatenate(outs, axis=1).astype(np.float32)
    return full.reshape(1, S, HID)



# revision 2
# speedup vs baseline: 1.2093x; 1.2093x over previous
"""Gemma2 sliding-window attention (B=1, S=4096, HID=3584, 16 Q heads / 8 KV heads,
HD=256, window 2047, tanh softcap 50) on 8 Trainium2 NeuronCores.

Sharding: tensor-parallel over heads with NO on-device collectives. Core c owns
Q heads (2c, 2c+1) and KV head c, and computes a full-shape PARTIAL of the
output projection restricted to its own 512 attention features:
    partial_c = attn[:, 512c:512c+512] @ w_o[:, 512c:512c+512].T   [S, HID] f32
The host sums the 8 partials (unshard of the sum-sharded output). This removes
the AllGather + serial o-proj tail of the previous design.

Per-core fused pipeline over 512-token tiles tt=0..7:
  A(tt): QKV projection (transposed for Q/K, straight for V) + NeoX RoPE.
  B(tt): sliding-window attention for query block tt (keys only need tiles
         <= tt, which are already computed). Tanh-softcap, no max-subtraction
         softmax, denominator via ones-row matmul, multiplicative boundary
         masks. o-proj chunks of block tt-1 are interleaved into the PV loop
         so the tensor engine never stalls on the activation engine.
All engines stay busy inside one pass; the only exposed tail is the last
o-proj block (~30 us).

PSUM (8 banks): psX bufs=3 shared by A's QKV groups, B's score tiles and C's
o-proj accumulators; psO bufs=3 for the PV accumulators; psVD bufs=2 shared by
A's V-projection groups and B's denominator row.
"""

import sys

if "/opt/trn_rl_repo" not in sys.path:
    sys.path.insert(0, "/opt/trn_rl_repo")

import numpy as np
import ml_dtypes

import concourse.bass as bass
import concourse.tile as tile
from concourse import bacc, mybir
from concourse.bass_utils import run_bass_kernel_spmd

# Problem constants (hardcoded per harness contract)
S = 4096
HID = 3584
NH, NKV, HD = 16, 8, 256
Q_SIZE = NH * HD          # 4096
SCALE = 256.0 ** -0.5     # 1/16
SOFTCAP = 50.0
WINDOW = 2048 - 1         # 2047
THETA = 10000.0

N_CORES = 8
QK_F = 2 * HD + HD        # 768 per-core transposed-qk features: [q_h0, q_h1, k]
KO = HID // 128           # 28 contraction subtiles for projections
TT = S // 512             # 8 token tiles of 512
HC = HID // 512           # 7 output-column chunks of 512
F32 = mybir.dt.float32
BF16 = mybir.dt.bfloat16

# Boundary-tile diagonal offsets (q0 - 128*kt). Interior iff 128 <= off <= 1536.
MASK_OFFS = [-384, -256, -128, 0, 1664, 1792, 1920, 2048]

_NC_CACHE = {}


def build_nc():
    nc = bacc.Bacc()

    hidT = nc.declare_dram_parameter("hidT", [HID, S], BF16, isOutput=False)
    wqkT = nc.declare_dram_parameter("wqkT", [HID, QK_F], BF16, isOutput=False)
    wvT = nc.declare_dram_parameter("wvT", [HID, HD], BF16, isOutput=False)
    woT = nc.declare_dram_parameter("woT", [4 * 128, HID], BF16, isOutput=False)
    cosT = nc.declare_dram_parameter("cosT", [128, S], F32, isOutput=False)
    sinT = nc.declare_dram_parameter("sinT", [128, S], F32, isOutput=False)
    masks = nc.declare_dram_parameter("masks", [8, 128, 512], BF16, isOutput=False)
    out = nc.declare_dram_parameter("out", [S, HID], F32, isOutput=True)

    hidT_r = hidT.rearrange("(ko p) t -> p ko t", p=128)
    wqkT_r = wqkT.rearrange("(ko p) f -> p ko f", p=128)
    wvT_r = wvT.rearrange("(ko p) d -> p ko d", p=128)
    woT_r = woT.rearrange("(fs p) h -> p fs h", p=128)
    masks_r = masks.rearrange("m p q -> p m q")

    with tile.TileContext(nc) as tc:
        with (
            tc.tile_pool(name="persist", bufs=1) as persist,
            tc.tile_pool(name="hidp", bufs=2) as hid_pool,
            tc.tile_pool(name="cs", bufs=2) as cs_pool,
            tc.tile_pool(name="qp", bufs=2) as q_pool,
            tc.tile_pool(name="rp", bufs=2) as rp_pool,
            tc.tile_pool(name="probs", bufs=6) as probs_pool,
            tc.tile_pool(name="aop", bufs=8) as ao_pool,
            tc.tile_pool(name="otp", bufs=3) as out_pool,
            tc.tile_pool(name="small", bufs=2) as small_pool,
            tc.tile_pool(name="psX", bufs=3, space="PSUM") as psX,
            tc.tile_pool(name="psO", bufs=3, space="PSUM") as psO,
            tc.tile_pool(name="psVD", bufs=2, space="PSUM") as psVD,
        ):
            # Persistent SBUF: weights, per-tile K/V, masks, ones
            wqk_sb = persist.tile([128, KO, QK_F], BF16, tag="wqk")
            # split the wqk load by 128-col slice so the first A group can
            # start as soon as its own slice lands
            for sl in range(6):
                nc.sync.dma_start(
                    wqk_sb[:, :, bass.ts(sl, 128)], wqkT_r[:, :, bass.ts(sl, 128)]
                )
            wv_sb = persist.tile([128, KO, HD], BF16, tag="wv")
            nc.sync.dma_start(wv_sb, wvT_r)
            mask_sb = persist.tile([128, 8, 512], BF16, tag="mask")
            nc.sync.dma_start(mask_sb, masks_r)
            wo_sb = persist.tile([128, 4, HID], BF16, tag="wo")
            nc.sync.dma_start(wo_sb, woT_r)
            ones_sb = persist.tile([128, 1], BF16, tag="ones")
            nc.vector.memset(ones_sb, 1.0)

            k_sb = [persist.tile([128, 2, 512], BF16, tag=f"k{t}", name=f"k{t}")
                    for t in range(TT)]
            v_sb = [persist.tile([128, 4, HD], BF16, tag=f"v{t}", name=f"v{t}")
                    for t in range(TT)]

            ao_store = {}

            def emit_A(tt):
                """QKV projection + RoPE for token tile tt. Returns q tile."""
                q_t = q_pool.tile([128, 4, 512], BF16, tag="q", name="q_t")
                cos_t = cs_pool.tile([128, 512], F32, tag="cos", name="cos_t")
                nc.sync.dma_start(cos_t, cosT[:, bass.ts(tt, 512)])
                sin_t = cs_pool.tile([128, 512], F32, tag="sin", name="sin_t")
                nc.sync.dma_start(sin_t, sinT[:, bass.ts(tt, 512)])
                for half in range(2):
                    t0 = tt * 512 + half * 256
                    csl = bass.ts(half, 256)
                    hid_h = hid_pool.tile([128, KO, 256], BF16, tag="hid",
                                          name="hid_h")
                    nc.sync.dma_start(hid_h, hidT_r[:, :, t0:t0 + 256])
                    for pair in range(3):
                        ps_a = psX.tile([128, 256], F32, tag="x", name="ps_a")
                        for ko in range(KO):
                            nc.tensor.matmul(
                                ps_a,
                                wqk_sb[:, ko, bass.ts(2 * pair, 128)],
                                hid_h[:, ko, :],
                                start=(ko == 0), stop=(ko == KO - 1),
                            )
                        ps_b = psX.tile([128, 256], F32, tag="x", name="ps_b")
                        for ko in range(KO):
                            nc.tensor.matmul(
                                ps_b,
                                wqk_sb[:, ko, bass.ts(2 * pair + 1, 128)],
                                hid_h[:, ko, :],
                                start=(ko == 0), stop=(ko == KO - 1),
                            )
                        if pair < 2:
                            d1 = q_t[:, 2 * pair, csl]
                            d2 = q_t[:, 2 * pair + 1, csl]
                        else:
                            d1 = k_sb[tt][:, 0, csl]
                            d2 = k_sb[tt][:, 1, csl]
                        t1 = rp_pool.tile([128, 256], F32, tag="rp", name="t1")
                        t2 = rp_pool.tile([128, 256], F32, tag="rp", name="t2")
                        nc.vector.tensor_mul(t1, ps_a, cos_t[:, csl])
                        nc.vector.tensor_mul(t2, ps_b, sin_t[:, csl])
                        nc.vector.tensor_sub(d1, t1, t2)
                        t3 = rp_pool.tile([128, 256], F32, tag="rp", name="t3")
                        t4 = rp_pool.tile([128, 256], F32, tag="rp", name="t4")
                        nc.vector.tensor_mul(t3, ps_b, cos_t[:, csl])
                        nc.vector.tensor_mul(t4, ps_a, sin_t[:, csl])
                        nc.vector.tensor_add(d2, t3, t4)
                    for j in range(2):
                        ps_v = psVD.tile([128, HD], F32, tag="vd", name="ps_v")
                        for ko in range(KO):
                            nc.tensor.matmul(
                                ps_v,
                                hid_h[:, ko, bass.ts(j, 128)],
                                wv_sb[:, ko, :],
                                start=(ko == 0), stop=(ko == KO - 1),
                            )
                        nc.scalar.copy(v_sb[tt][:, half * 2 + j, :], ps_v)
                return q_t

            def emit_C_chunks(qb):
                """o-proj partial for query block qb: 28 chunk generators."""
                ao_h0 = ao_store.pop((qb, 0))
                ao_h1 = ao_store.pop((qb, 1))
                aos = [ao_h0[0], ao_h0[1], ao_h1[0], ao_h1[1]]
                for tsub in range(4):
                    for hc in range(HC):
                        ps = psX.tile([128, 512], F32, tag="x", name="psC")
                        for fs in range(4):
                            nc.tensor.matmul(
                                ps,
                                aos[fs][:, bass.ts(tsub, 128)],
                                wo_sb[:, fs, bass.ts(hc, 512)],
                                start=(fs == 0), stop=(fs == 3),
                                skip_group_check=True,
                            )
                        ot = out_pool.tile([128, 512], F32, tag="ot", name="ot")
                        nc.scalar.copy(ot, ps)
                        r0 = qb * 512 + tsub * 128
                        nc.sync.dma_start(
                            out[r0:r0 + 128, bass.ts(hc, 512)], ot
                        )
                        yield

            def emit_B(qb, q_t, cgen):
                """Attention for query block qb, o-proj chunks interleaved."""
                q0 = qb * 512
                kts = list(range(max(0, 4 * qb - 16), 4 * qb + 4))
                n = len(kts)
                for h in range(2):
                    po0 = psO.tile([128, 512], F32, tag="po", name="po0")
                    po1 = psO.tile([128, 512], F32, tag="po", name="po1")
                    pden = psVD.tile([1, 512], F32, tag="vd", name="pden")
                    probs = {}

                    def scores(i, h=h, probs=probs):
                        kt = kts[i]
                        ttk, ksub = kt // 4, kt % 4
                        ksl = bass.ts(ksub, 128)
                        ps = psX.tile([128, 512], F32, tag="x", name="ps_s")
                        nc.tensor.matmul(
                            ps, k_sb[ttk][:, 0, ksl], q_t[:, 2 * h, :],
                            start=True, stop=False,
                        )
                        nc.tensor.matmul(
                            ps, k_sb[ttk][:, 1, ksl], q_t[:, 2 * h + 1, :],
                            start=False, stop=True,
                        )
                        pt = probs_pool.tile([128, 512], BF16, tag="pt",
                                             name="pt")
                        nc.scalar.activation(
                            ps, ps, mybir.ActivationFunctionType.Tanh,
                            scale=SCALE / SOFTCAP,
                        )
                        nc.scalar.activation(
                            pt, ps, mybir.ActivationFunctionType.Exp,
                            scale=SOFTCAP,
                        )
                        off = q0 - 128 * kt
                        if not (128 <= off <= 1536):
                            mi = MASK_OFFS.index(off)
                            nc.vector.tensor_mul(pt, pt, mask_sb[:, mi, :])
                        probs[i] = pt

                    def av(i, probs=probs, po0=po0, po1=po1, pden=pden):
                        kt = kts[i]
                        ttk, ksub = kt // 4, kt % 4
                        pt = probs.pop(i)
                        st, sp = (i == 0), (i == n - 1)
                        nc.tensor.matmul(po0, v_sb[ttk][:, ksub, 0:128], pt,
                                         start=st, stop=sp,
                                         skip_group_check=True)
                        nc.tensor.matmul(po1, v_sb[ttk][:, ksub, 128:256], pt,
                                         start=st, stop=sp,
                                         skip_group_check=True)
                        nc.tensor.matmul(pden, ones_sb, pt,
                                         start=st, stop=sp,
                                         skip_group_check=True)

                    LOOK = 3
                    next(cgen, None)
                    for i in range(min(LOOK, n)):
                        scores(i)
                    for i in range(n):
                        if i + LOOK < n:
                            scores(i + LOOK)
                        av(i)
                        next(cgen, None)

                    recip = small_pool.tile([1, 512], F32, tag="recip",
                                            name="recip")
                    nc.vector.reciprocal(recip, pden)
                    rb = small_pool.tile([128, 512], F32, tag="rb", name="rb")
                    nc.gpsimd.partition_broadcast(rb, recip)
                    ao0 = ao_pool.tile([128, 512], BF16, tag="ao", name="ao0")
                    ao1 = ao_pool.tile([128, 512], BF16, tag="ao", name="ao1")
                    nc.vector.tensor_mul(ao0, po0, rb)
                    nc.vector.tensor_mul(ao1, po1, rb)
                    ao_store[(qb, h)] = (ao0, ao1)

            for tt in range(TT):
                q_t = emit_A(tt)
                cgen = emit_C_chunks(tt - 1) if tt > 0 else iter(())
                emit_B(tt, q_t, cgen)
                for _ in cgen:
                    pass
            for _ in emit_C_chunks(TT - 1):
                pass

    nc.compile()
    return nc


def get_nc():
    if "nc" not in _NC_CACHE:
        _NC_CACHE["nc"] = build_nc()
    return _NC_CACHE["nc"]


def prep_in_maps(inputs):
    bf16 = ml_dtypes.bfloat16
    hs = np.asarray(inputs["hidden_states"], dtype=np.float32)
    pos = np.asarray(inputs["position_ids"]).reshape(-1).astype(np.float64)
    w_qkv = np.asarray(inputs["w_qkv"], dtype=np.float32)
    w_o = np.asarray(inputs["w_o"], dtype=np.float32)

    hidT = np.ascontiguousarray(hs.reshape(S, HID).T).astype(bf16)

    inv_freq = 1.0 / (THETA ** (np.arange(HD // 2, dtype=np.float64) * 2.0 / HD))
    ang = inv_freq[:, None] * pos[None, :]
    cosT = np.cos(ang).astype(np.float32)
    sinT = np.sin(ang).astype(np.float32)

    kk = np.arange(128)[:, None]
    qq = np.arange(512)[None, :]
    masks = np.stack(
        [((qq - kk + o >= 0) & (qq - kk + o <= WINDOW)) for o in MASK_OFFS]
    ).astype(bf16)

    in_maps = []
    for c in range(N_CORES):
        wq = w_qkv[512 * c:512 * (c + 1)]
        wk = w_qkv[Q_SIZE + HD * c:Q_SIZE + HD * (c + 1)]
        wv = w_qkv[Q_SIZE + NKV * HD + HD * c:Q_SIZE + NKV * HD + HD * (c + 1)]
        wqkT = np.ascontiguousarray(np.concatenate([wq, wk], 0).T).astype(bf16)
        wvT = np.ascontiguousarray(wv.T).astype(bf16)
        woT = np.ascontiguousarray(w_o[:, 512 * c:512 * (c + 1)].T).astype(bf16)
        in_maps.append(
            dict(hidT=hidT, wqkT=wqkT, wvT=wvT, woT=woT,
                 cosT=cosT, sinT=sinT, masks=masks)
        )
    return in_maps


def run(inputs, **kwargs):
    nc = get_nc()
    in_maps = prep_in_maps(inputs)
    return run_bass_kernel_spmd(nc, in_maps, list(range(N_CORES)), **kwargs)


def gather_results(res):
    """Sum the 8 full-shape partials (unshard of sum-sharded output)."""
    acc = np.zeros((S, HID), dtype=np.float64)
    for c in range(N_CORES):
        acc += np.asarray(res.results[c]["out"], dtype=np.float64)
    return acc.astype(np.float32).reshape(1, S, HID)


def kernel(**inputs):
    res = run(inputs)
    return gather_results(res)
